# revision 21
# baseline (speedup 1.0000x reference)
"""Trainium2 Bass kernel for dynamic adaptive-pooling depthwise conv.

Problem: x [16,128,192,192] f32. Per-sample selector head (global mean ->
MLP -> softmax over K=2) mixes a bank of K depthwise 3x3 kernels; then a
per-(sample,channel) 3x3 depthwise conv + bias.

Strategy (8 NeuronCores, data-parallel over batch, 2 samples/core),
single-read design: x is read from HBM exactly once. v3:
  - PE (diag-stationary matmuls): taps {0,2,3,5,6,8} always, tap 7 on
    even groups only (odd groups get it on DVE as 4x tensor_scalar + 2x
    tensor_tensor) -- balances PE vs DVE busy time.
  - ACT: f32->bf16 cast w/ fused per-channel partial sums + tap-1 map.
  - DVE: fused tap-4 STT chain, tap-7 split share, PSUM eviction as two
    3D STTs per 3-pair group (3-bank PSUM tiles, 2 groups in flight so
    the PE never stalls on bank reuse), halo-row copies (4x bf16).
  - Pool/GPSIMD: memsets only (its SW ops are slow and steal DVE SBUF
    ports -- measured).
kernel(**inputs) takes FULL inputs, shards batch over 8 cores, returns
FULL output. Self-contained: hardcodes all shapes.
"""
import numpy as np

B, C, H, W = 16, 128, 192, 192
NCORES = 8
BC = B // NCORES          # samples per core
RS = 196                  # padded row stride (192 img + 4 pad cols, even)
LEAD = 2                  # leading pad elems (keeps tap offsets 4B-aligned)
R = 24                    # output rows per conv tile
NT = H // R               # conv tiles per sample (8)
NPAIR = R // 2            # psum row-pairs per conv tile (12)
XT_FLAT = LEAD + (R + 2) * RS + 2
HR = R // 2               # rows per f32 load chunk (12)
GRP = 3                   # row-pairs per psum group (3 banks per tile)
BANK = 512                # f32 elems per PSUM bank per partition
PE_TAPS = [0, 2, 3, 5, 6, 8]   # t1 on ACT; t4 on DVE; t7 split PE/DVE

_cache = {}


def _build():
    from concourse import bacc, mybir
    from concourse.tile import TileContext

    f32 = mybir.dt.float32
    bf16 = mybir.dt.bfloat16
    AF = mybir.ActivationFunctionType
    ALU = mybir.AluOpType
    AX = mybir.AxisListType

    nc = bacc.Bacc()
    x_ext = nc.declare_dram_parameter("x", [BC, C, H, W], f32, isOutput=False)
    out_ext = nc.declare_dram_parameter("out", [BC, C, H, W], f32, isOutput=True)
    w1T_ext = nc.declare_dram_parameter("w1T", [C, 32], f32, isOutput=False)
    b1_ext = nc.declare_dram_parameter("b1c", [32, 1], f32, isOutput=False)
    w2T_ext = nc.declare_dram_parameter("w2T", [32, 2], f32, isOutput=False)
    b2_ext = nc.declare_dram_parameter("b2c", [2, 1], f32, isOutput=False)
    ones2_ext = nc.declare_dram_parameter("ones2", [2, C], f32, isOutput=False)
    bankT_ext = nc.declare_dram_parameter("bankT", [2, 9 * C], f32, isOutput=False)
    bias_ext = nc.declare_dram_parameter("biasc", [C, 1], f32, isOutput=False)
    ident_ext = nc.declare_dram_parameter("ident", [C, C], f32, isOutput=False)

    with TileContext(nc) as tc:
        with (
            tc.tile_pool(name="consts", bufs=1) as consts,
            tc.tile_pool(name="stg", bufs=4) as stgp,
            tc.tile_pool(name="xt", bufs=NT + 1) as xcp,
            tc.tile_pool(name="stat", bufs=2) as statp,
            tc.tile_pool(name="sel", bufs=2) as selp,
            tc.tile_pool(name="diag", bufs=2) as diagp,
            tc.tile_pool(name="map", bufs=4) as mapp,
            tc.tile_pool(name="outp", bufs=3) as outp,
            tc.tile_pool(name="psc", bufs=2, space="PSUM") as psc,
            tc.tile_pool(name="pss", bufs=2, space="PSUM") as pss,
        ):
            def cload(shape, ext, tag):
                # consts ride the store ring (idle at start) so the x-chunk
                # loads own the sync ring from t=0
                t = consts.tile(shape, f32, tag=tag)
                nc.scalar.dma_start(out=t, in_=ext[:, :])
                return t
            w1T_sb = cload([C, 32], w1T_ext, "c_w1T")
            b1_sb = cload([32, 1], b1_ext, "c_b1")
            w2T_sb = cload([32, 2], w2T_ext, "c_w2T")
            b2_sb = cload([2, 1], b2_ext, "c_b2")
            ones2_sb = cload([2, C], ones2_ext, "c_ones2")
            bankT_sb = cload([2, 9 * C], bankT_ext, "c_bankT")
            bias_sb = cload([C, 1], bias_ext, "c_bias")
            ident_sb = cload([C, C], ident_ext, "c_ident")
            identbf = consts.tile([C, C], bf16, tag="c_identbf")
            nc.vector.tensor_copy(identbf, ident_sb)

            def load_tile(b, ti, partials):
                """Load+cast one 24-row tile (no halo rows); fused sums."""
                r0 = ti * R
                xt = xcp.tile([C, XT_FLAT], bf16, tag="xt")
                xt3 = xt[:, LEAD:LEAD + (R + 2) * RS].rearrange(
                    "p (r c) -> p r c", c=RS)
                nc.gpsimd.memset(xt[:, 0:LEAD], 0.0)
                nc.gpsimd.memset(xt[:, XT_FLAT - 2:XT_FLAT], 0.0)
                nc.gpsimd.memset(xt3[:, :, 192:196], 0.0)
                if ti == 0:
                    nc.gpsimd.memset(xt3[:, 0:1, 0:192], 0.0)
                if ti == NT - 1:
                    nc.gpsimd.memset(xt3[:, R + 1:R + 2, 0:192], 0.0)
                for half in range(2):
                    qa = r0 + half * HR
                    stg = stgp.tile([C, HR * W], f32, tag="stg")
                    ring = nc.scalar if (b == 0 and half == 1) else nc.sync
                    ring.dma_start(out=stg, in_=x_ext[b][:, qa:qa + HR, :])
                    s3 = stg.rearrange("p (r c) -> p r c", c=W)
                    pc = partials[:, 2 * ti + half:2 * ti + half + 1]
                    nc.scalar.activation(
                        xt3[:, 1 + half * HR:1 + (half + 1) * HR, 0:192],
                        s3, AF.Copy, accum_out=pc)
                return xt

            def halo_fix(xts, ti):
                """Copy halo rows between tiles ti-1/ti (DVE 4x bf16)."""
                a, b_ = xts[ti - 1], xts[ti]
                a3 = a[:, LEAD:LEAD + (R + 2) * RS].rearrange(
                    "p (r c) -> p r c", c=RS)
                b3 = b_[:, LEAD:LEAD + (R + 2) * RS].rearrange(
                    "p (r c) -> p r c", c=RS)
                nc.vector.tensor_copy(a3[:, R + 1:R + 2, 0:192],
                                      b3[:, 1:2, 0:192])
                nc.vector.tensor_copy(b3[:, 0:1, 0:192],
                                      a3[:, R:R + 1, 0:192])

            def selector(b, partials, warm_src):
                def dummy(n):
                    for _ in range(n):
                        dps = pss.tile([C, 512], f32, tag="selps")
                        nc.tensor.matmul(dps, identbf[:, :],
                                         warm_src[:, LEAD:LEAD + 512],
                                         start=True, stop=True)
                pooled = statp.tile([C, 1], f32, tag="pooled")
                nc.vector.reduce_sum(pooled, partials, axis=AX.X)
                hA = pss.tile([32, 1], f32, tag="selps")
                nc.tensor.matmul(hA, w1T_sb[:, :], pooled[:, :], start=True, stop=True)
                if warm_src is not None:
                    dummy(3)
                hs = selp.tile([32, 1], f32, tag="hs")
                nc.scalar.activation(hs, hA, AF.Relu, bias=b1_sb[:, :])
                lB = pss.tile([2, 1], f32, tag="selps")
                nc.tensor.matmul(lB, w2T_sb[:, :], hs[:, :], start=True, stop=True)
                if warm_src is not None:
                    dummy(3)
                es = selp.tile([2, 1], f32, tag="es")
                nc.scalar.activation(es, lB, AF.Exp, bias=b2_sb[:, :])
                Sps = pss.tile([C, 1], f32, tag="selps")
                nc.tensor.matmul(Sps, ones2_sb[:, :], es[:, :], start=True, stop=True)
                if warm_src is not None:
                    dummy(3)
                invS = selp.tile([C, 1], f32, tag="invS")
                nc.vector.reciprocal(invS, Sps)
                cwps = pss.tile([C, 9], f32, tag="selps")
                for t in range(9):
                    nc.tensor.matmul(cwps[:, t:t + 1],
                                     bankT_sb[:, t * C:(t + 1) * C], es[:, :],
                                     start=True, stop=True)
                cw = selp.tile([C, 9], f32, tag="cw")
                nc.vector.tensor_scalar(cw, cwps, invS[:, :], None, ALU.mult)
                diag = diagp.tile([C, 9 * C], bf16, tag="diag")
                for t in PE_TAPS + [7]:
                    # diag builds on ACT (has slack; frees DVE)
                    nc.scalar.activation(diag[:, t * C:(t + 1) * C], ident_sb,
                                         AF.Copy, scale=cw[:, t:t + 1])
                return diag, cw

            def conv_tile(b, ti, xt, diag, cw):
                r0 = ti * R
                ot = None
                for ga in range(0, NPAIR, GRP):
                    if ga % (2 * GRP) == 0:
                        # half-tile staging: store every 2 groups (12 rows)
                        # for a shorter drain tail and smaller SBUF tiles
                        ot = outp.tile([C, R * W // 2], f32, tag="ot")
                        ot4 = ot.rearrange("p (j r c) -> p j r c", r=2, c=192)
                        oj = ga
                    glen = 2 * GRP * RS
                    t7_pe = (ga // GRP) % 2 == 0
                    c1 = LEAD + 2 * ga * RS
                    c4 = LEAD + (2 * ga + 1) * RS
                    c7 = LEAD + (2 * ga + 2) * RS
                    # t1 map on ACT, then fused t4 chain on DVE
                    m1 = mapp.tile([C, 2 * GRP * RS], bf16, tag="m1")
                    nc.scalar.activation(m1, xt[:, c1:c1 + glen],
                                         AF.Copy, scale=cw[:, 1:2])
                    ts = mapp.tile([C, 2 * GRP * RS], bf16, tag="ts")
                    nc.vector.scalar_tensor_tensor(
                        ts, xt[:, c4:c4 + glen], cw[:, 4:5], m1,
                        ALU.mult, ALU.add)
                    if not t7_pe:
                        m7 = mapp.tile([C, 2 * GRP * RS], bf16, tag="m7")
                        nc.vector.tensor_scalar(m7, xt[:, c7:c7 + glen],
                                                cw[:, 7:8], None, ALU.mult)
                        nc.vector.tensor_tensor(ts, ts, m7, ALU.add)
                    # PE taps into one 3-bank PSUM tile (pair j -> bank j)
                    pt = psc.tile([C, GRP * BANK], f32, tag="pt")
                    taps = PE_TAPS + [7] if t7_pe else PE_TAPS
                    for t in taps:
                        dh, dw = divmod(t, 3)
                        for j in range(GRP):
                            s = LEAD + (2 * (ga + j) + dh) * RS + dw - 1
                            nc.tensor.matmul(
                                pt[:, j * BANK:j * BANK + 2 * RS],
                                diag[:, t * C:(t + 1) * C],
                                xt[:, s:s + 2 * RS],
                                start=(t == taps[0]), stop=(t == taps[-1]))
                    # evict: two 3D STTs per group (row 0 / row 1)
                    ptb = pt.rearrange("p (b x) -> p b x", b=GRP)
                    ts4 = ts.rearrange("p (j r c) -> p j r c", r=2, c=RS)
                    for r in range(2):
                        ov = ot4[:, ga - oj:ga - oj + GRP, r, :]
                        nc.vector.scalar_tensor_tensor(
                            ov, ptb[:, :, r * RS:r * RS + 192], bias_sb[:, :],
                            ts4[:, :, r, 0:192], ALU.add, ALU.add)
                    if (ga + GRP) % (2 * GRP) == 0:
                        q0 = r0 + 2 * oj
                        nc.scalar.dma_start(
                            out=out_ext[b][:, q0:q0 + R // 2, :], in_=ot)

            # ---- schedule ----
            partials0 = statp.tile([C, 2 * NT], f32, tag="part0")
            xts0 = [None] * NT
            for ti in range(NT):
                xts0[ti] = load_tile(0, ti, partials0)
                if ti > 0:
                    halo_fix(xts0, ti)
            diag0, cw0 = selector(0, partials0, xts0[NT - 1])
            partials1 = statp.tile([C, 2 * NT], f32, tag="part1")
            xts1 = [None] * NT
            for ti in range(NT):
                xts1[ti] = load_tile(1, ti, partials1)
                if ti > 0:
                    halo_fix(xts1, ti)
                if ti == NT - 1:
                    # emit sel(1) before conv(0,last) so its ACT/DVE chain
                    # overlaps the last sample-0 conv tile's matmuls
                    diag1, cw1 = selector(1, partials1, None)
                conv_tile(0, ti, xts0[ti], diag0, cw0)
            for ti in range(NT):
                conv_tile(1, ti, xts1[ti], diag1, cw1)

    nc.finalize()
    return nc


def kernel(x, w1, b1, w2, b2, weight_bank, bias):
    from concourse.bass_utils import run_bass_kernel_spmd

    x = np.ascontiguousarray(np.asarray(x, dtype=np.float32))
    w1 = np.asarray(w1, dtype=np.float32)
    b1 = np.asarray(b1, dtype=np.float32)
    w2 = np.asarray(w2, dtype=np.float32)
    b2 = np.asarray(b2, dtype=np.float32)
    weight_bank = np.asarray(weight_bank, dtype=np.float32)
    bias = np.asarray(bias, dtype=np.float32)

    if "nc" not in _cache:
        _cache["nc"] = _build()
    nc = _cache["nc"]

    w1T_s = np.ascontiguousarray(w1.T / float(H * W))
    w2T = np.ascontiguousarray(w2.T)
    ones2 = np.ones((2, C), np.float32)
    bankT = np.ascontiguousarray(
        np.transpose(weight_bank.reshape(2, C, 9), (0, 2, 1)).reshape(2, 9 * C))
    ident = np.eye(C, dtype=np.float32)
    common = {
        "w1T": w1T_s, "b1c": np.ascontiguousarray(b1.reshape(32, 1)),
        "w2T": w2T, "b2c": np.ascontiguousarray(b2.reshape(2, 1)),
        "ones2": ones2, "bankT": bankT,
        "biasc": np.ascontiguousarray(bias.reshape(C, 1)), "ident": ident,
    }
    in_maps = [dict(common, x=x[i * BC:(i + 1) * BC]) for i in range(NCORES)]
    res = run_bass_kernel_spmd(nc, in_maps, core_ids=list(range(NCORES)))
    _cache["last_result"] = res
    out = np.concatenate([np.asarray(res.results[i]["out"]) for i in range(NCORES)],
                         axis=0)
    return out


# revision 22
# speedup vs baseline: 1.0280x; 1.0280x over previous
"""Trainium2 Bass kernel for dynamic adaptive-pooling depthwise conv.

Problem: x [16,128,192,192] f32. Per-sample selector head (global mean ->
MLP -> softmax over K=2) mixes a bank of K depthwise 3x3 kernels; then a
per-(sample,channel) 3x3 depthwise conv + bias.

Strategy (8 NeuronCores, data-parallel over batch, 2 samples/core),
single-read design: x is read from HBM exactly once. v3:
  - PE (diag-stationary matmuls): taps {0,2,3,5,6,8} always, tap 7 on
    even groups only (odd groups get it on DVE as 4x tensor_scalar + 2x
    tensor_tensor) -- balances PE vs DVE busy time.
  - ACT: f32->bf16 cast w/ fused per-channel partial sums + tap-1 map.
  - DVE: fused tap-4 STT chain, tap-7 split share, PSUM eviction as two
    3D STTs per 3-pair group (3-bank PSUM tiles, 2 groups in flight so
    the PE never stalls on bank reuse), halo-row copies (4x bf16).
  - Pool/GPSIMD: memsets only (its SW ops are slow and steal DVE SBUF
    ports -- measured).
kernel(**inputs) takes FULL inputs, shards batch over 8 cores, returns
FULL output. Self-contained: hardcodes all shapes.
"""
import numpy as np

B, C, H, W = 16, 128, 192, 192
NCORES = 8
BC = B // NCORES          # samples per core
RS = 196                  # padded row stride (192 img + 4 pad cols, even)
LEAD = 2                  # leading pad elems (keeps tap offsets 4B-aligned)
R = 24                    # output rows per conv tile
NT = H // R               # conv tiles per sample (8)
NPAIR = R // 2            # psum row-pairs per conv tile (12)
XT_FLAT = LEAD + (R + 2) * RS + 2
HR = R // 2               # rows per f32 load chunk (12)
GRP = 3                   # row-pairs per psum group (3 banks per tile)
BANK = 512                # f32 elems per PSUM bank per partition
PE_TAPS = [0, 2, 3, 5, 6, 8]   # t1 on ACT; t4 on DVE; t7 split PE/DVE

_cache = {}


def _build():
    from concourse import bacc, mybir
    from concourse.tile import TileContext

    f32 = mybir.dt.float32
    bf16 = mybir.dt.bfloat16
    AF = mybir.ActivationFunctionType
    ALU = mybir.AluOpType
    AX = mybir.AxisListType

    nc = bacc.Bacc()
    x_ext = nc.declare_dram_parameter("x", [BC, C, H, W], f32, isOutput=False)
    out_ext = nc.declare_dram_parameter("out", [BC, C, H, W], f32, isOutput=True)
    w1T_ext = nc.declare_dram_parameter("w1T", [C, 32], f32, isOutput=False)
    b1_ext = nc.declare_dram_parameter("b1c", [32, 1], f32, isOutput=False)
    w2T_ext = nc.declare_dram_parameter("w2T", [32, 2], f32, isOutput=False)
    b2_ext = nc.declare_dram_parameter("b2c", [2, 1], f32, isOutput=False)
    ones2_ext = nc.declare_dram_parameter("ones2", [2, C], f32, isOutput=False)
    bankT_ext = nc.declare_dram_parameter("bankT", [2, 9 * C], f32, isOutput=False)
    bias_ext = nc.declare_dram_parameter("biasc", [C, 1], f32, isOutput=False)
    ident_ext = nc.declare_dram_parameter("ident", [C, C], f32, isOutput=False)

    with TileContext(nc) as tc:
        with (
            tc.tile_pool(name="consts", bufs=1) as consts,
            tc.tile_pool(name="stg", bufs=4) as stgp,
            tc.tile_pool(name="xt", bufs=NT + 1) as xcp,
            tc.tile_pool(name="stat", bufs=2) as statp,
            tc.tile_pool(name="sel", bufs=2) as selp,
            tc.tile_pool(name="diag", bufs=2) as diagp,
            tc.tile_pool(name="map", bufs=3) as mapp,
            tc.tile_pool(name="outp", bufs=2) as outp,
            tc.tile_pool(name="psc", bufs=2, space="PSUM") as psc,
            tc.tile_pool(name="pss", bufs=2, space="PSUM") as pss,
        ):
            def cload(shape, ext, tag):
                t = consts.tile(shape, f32, tag=tag)
                nc.sync.dma_start(out=t, in_=ext[:, :])
                return t
            w1T_sb = cload([C, 32], w1T_ext, "c_w1T")
            b1_sb = cload([32, 1], b1_ext, "c_b1")
            w2T_sb = cload([32, 2], w2T_ext, "c_w2T")
            b2_sb = cload([2, 1], b2_ext, "c_b2")
            ones2_sb = cload([2, C], ones2_ext, "c_ones2")
            bankT_sb = cload([2, 9 * C], bankT_ext, "c_bankT")
            bias_sb = cload([C, 1], bias_ext, "c_bias")
            ident_sb = cload([C, C], ident_ext, "c_ident")
            identbf = consts.tile([C, C], bf16, tag="c_identbf")
            nc.vector.tensor_copy(identbf, ident_sb)

            def load_tile(b, ti, partials):
                """Load+cast one 24-row tile (no halo rows); fused sums."""
                r0 = ti * R
                xt = xcp.tile([C, XT_FLAT], bf16, tag="xt")
                xt3 = xt[:, LEAD:LEAD + (R + 2) * RS].rearrange(
                    "p (r c) -> p r c", c=RS)
                nc.gpsimd.memset(xt[:, 0:LEAD], 0.0)
                nc.gpsimd.memset(xt[:, XT_FLAT - 2:XT_FLAT], 0.0)
                nc.gpsimd.memset(xt3[:, :, 192:196], 0.0)
                if ti == 0:
                    nc.gpsimd.memset(xt3[:, 0:1, 0:192], 0.0)
                if ti == NT - 1:
                    nc.gpsimd.memset(xt3[:, R + 1:R + 2, 0:192], 0.0)
                for half in range(2):
                    qa = r0 + half * HR
                    stg = stgp.tile([C, HR * W], f32, tag="stg")
                    nc.sync.dma_start(out=stg, in_=x_ext[b][:, qa:qa + HR, :])
                    s3 = stg.rearrange("p (r c) -> p r c", c=W)
                    pc = partials[:, 2 * ti + half:2 * ti + half + 1]
                    nc.scalar.activation(
                        xt3[:, 1 + half * HR:1 + (half + 1) * HR, 0:192],
                        s3, AF.Copy, accum_out=pc)
                return xt

            def halo_fix(xts, ti):
                """Copy halo rows between tiles ti-1/ti (DVE 4x bf16)."""
                a, b_ = xts[ti - 1], xts[ti]
                a3 = a[:, LEAD:LEAD + (R + 2) * RS].rearrange(
                    "p (r c) -> p r c", c=RS)
                b3 = b_[:, LEAD:LEAD + (R + 2) * RS].rearrange(
                    "p (r c) -> p r c", c=RS)
                nc.vector.tensor_copy(a3[:, R + 1:R + 2, 0:192],
                                      b3[:, 1:2, 0:192])
                nc.vector.tensor_copy(b3[:, 0:1, 0:192],
                                      a3[:, R:R + 1, 0:192])

            def selector(b, partials, warm_src):
                def dummy(n):
                    for _ in range(n):
                        dps = pss.tile([C, 512], f32, tag="selps")
                        nc.tensor.matmul(dps, identbf[:, :],
                                         warm_src[:, LEAD:LEAD + 512],
                                         start=True, stop=True)
                pooled = statp.tile([C, 1], f32, tag="pooled")
                nc.vector.reduce_sum(pooled, partials, axis=AX.X)
                hA = pss.tile([32, 1], f32, tag="selps")
                nc.tensor.matmul(hA, w1T_sb[:, :], pooled[:, :], start=True, stop=True)
                if warm_src is not None:
                    dummy(4)
                hs = selp.tile([32, 1], f32, tag="hs")
                nc.scalar.activation(hs, hA, AF.Relu, bias=b1_sb[:, :])
                lB = pss.tile([2, 1], f32, tag="selps")
                nc.tensor.matmul(lB, w2T_sb[:, :], hs[:, :], start=True, stop=True)
                if warm_src is not None:
                    dummy(4)
                es = selp.tile([2, 1], f32, tag="es")
                nc.scalar.activation(es, lB, AF.Exp, bias=b2_sb[:, :])
                Sps = pss.tile([C, 1], f32, tag="selps")
                nc.tensor.matmul(Sps, ones2_sb[:, :], es[:, :], start=True, stop=True)
                if warm_src is not None:
                    dummy(4)
                invS = selp.tile([C, 1], f32, tag="invS")
                nc.vector.reciprocal(invS, Sps)
                cwps = pss.tile([C, 9], f32, tag="selps")
                for t in range(9):
                    nc.tensor.matmul(cwps[:, t:t + 1],
                                     bankT_sb[:, t * C:(t + 1) * C], es[:, :],
                                     start=True, stop=True)
                cw = selp.tile([C, 9], f32, tag="cw")
                nc.vector.tensor_scalar(cw, cwps, invS[:, :], None, ALU.mult)
                diag = diagp.tile([C, 9 * C], bf16, tag="diag")
                for t in PE_TAPS + [7]:
                    nc.vector.tensor_scalar(diag[:, t * C:(t + 1) * C], ident_sb,
                                            cw[:, t:t + 1], None, ALU.mult)
                return diag, cw

            gctr = [0]

            def conv_tile(b, ti, xt, diag, cw):
                r0 = ti * R
                ot = outp.tile([C, R * W], f32, tag="ot")
                ot4 = ot.rearrange("p (j r c) -> p j r c", r=2, c=192)
                for ga in range(0, NPAIR, GRP):
                    glen = 2 * GRP * RS
                    gi = gctr[0]; gctr[0] += 1
                    # style: t7 on PE for 20 of every 32 "even" slots; the
                    # other even slots and all odd slots take t7 as an ACT
                    # map merged on DVE (rebalances PE->ACT/DVE)
                    t7_pe = (gi % 2 == 0) and (gi % 16 not in (0, 6))
                    c1 = LEAD + 2 * ga * RS
                    c4 = LEAD + (2 * ga + 1) * RS
                    c7 = LEAD + (2 * ga + 2) * RS
                    # t1 map on ACT, then fused t4 chain on DVE
                    m1 = mapp.tile([C, 2 * GRP * RS], bf16, tag="m1")
                    nc.scalar.activation(m1, xt[:, c1:c1 + glen],
                                         AF.Copy, scale=cw[:, 1:2])
                    ts = mapp.tile([C, 2 * GRP * RS], bf16, tag="ts")
                    nc.vector.scalar_tensor_tensor(
                        ts, xt[:, c4:c4 + glen], cw[:, 4:5], m1,
                        ALU.mult, ALU.add)
                    if not t7_pe:
                        m7 = mapp.tile([C, 2 * GRP * RS], bf16, tag="m7")
                        nc.scalar.activation(m7, xt[:, c7:c7 + glen],
                                             AF.Copy, scale=cw[:, 7:8])
                        nc.vector.tensor_tensor(ts, ts, m7, ALU.add)
                    # PE taps into one 3-bank PSUM tile (pair j -> bank j)
                    pt = psc.tile([C, GRP * BANK], f32, tag="pt")
                    taps = PE_TAPS + [7] if t7_pe else PE_TAPS
                    for t in taps:
                        dh, dw = divmod(t, 3)
                        for j in range(GRP):
                            s = LEAD + (2 * (ga + j) + dh) * RS + dw - 1
                            nc.tensor.matmul(
                                pt[:, j * BANK:j * BANK + 2 * RS],
                                diag[:, t * C:(t + 1) * C],
                                xt[:, s:s + 2 * RS],
                                start=(t == taps[0]), stop=(t == taps[-1]))
                    # evict: two 3D STTs per group (row 0 / row 1)
                    ptb = pt.rearrange("p (b x) -> p b x", b=GRP)
                    ts4 = ts.rearrange("p (j r c) -> p j r c", r=2, c=RS)
                    for r in range(2):
                        ov = ot4[:, ga:ga + GRP, r, :]
                        nc.vector.scalar_tensor_tensor(
                            ov, ptb[:, :, r * RS:r * RS + 192], bias_sb[:, :],
                            ts4[:, :, r, 0:192], ALU.add, ALU.add)
                nc.scalar.dma_start(out=out_ext[b][:, r0:r0 + R, :], in_=ot)

            # ---- schedule ----
            partials0 = statp.tile([C, 2 * NT], f32, tag="part0")
            xts0 = [None] * NT
            for ti in range(NT):
                xts0[ti] = load_tile(0, ti, partials0)
                if ti > 0:
                    halo_fix(xts0, ti)
            diag0, cw0 = selector(0, partials0, xts0[NT - 1])
            partials1 = statp.tile([C, 2 * NT], f32, tag="part1")
            xts1 = [None] * NT
            for ti in range(NT):
                xts1[ti] = load_tile(1, ti, partials1)
                if ti > 0:
                    halo_fix(xts1, ti)
                if ti == NT - 1:
                    # emit sel(1) before conv(0,last) so its ACT/DVE chain
                    # overlaps the last sample-0 conv tile's matmuls
                    diag1, cw1 = selector(1, partials1, None)
                conv_tile(0, ti, xts0[ti], diag0, cw0)
            for ti in range(NT):
                conv_tile(1, ti, xts1[ti], diag1, cw1)

    nc.finalize()
    return nc


def kernel(x, w1, b1, w2, b2, weight_bank, bias):
    from concourse.bass_utils import run_bass_kernel_spmd

    x = np.ascontiguousarray(np.asarray(x, dtype=np.float32))
    w1 = np.asarray(w1, dtype=np.float32)
    b1 = np.asarray(b1, dtype=np.float32)
    w2 = np.asarray(w2, dtype=np.float32)
    b2 = np.asarray(b2, dtype=np.float32)
    weight_bank = np.asarray(weight_bank, dtype=np.float32)
    bias = np.asarray(bias, dtype=np.float32)

    if "nc" not in _cache:
        _cache["nc"] = _build()
    nc = _cache["nc"]

    w1T_s = np.ascontiguousarray(w1.T / float(H * W))
    w2T = np.ascontiguousarray(w2.T)
    ones2 = np.ones((2, C), np.float32)
    bankT = np.ascontiguousarray(
        np.transpose(weight_bank.reshape(2, C, 9), (0, 2, 1)).reshape(2, 9 * C))
    ident = np.eye(C, dtype=np.float32)
    common = {
        "w1T": w1T_s, "b1c": np.ascontiguousarray(b1.reshape(32, 1)),
        "w2T": w2T, "b2c": np.ascontiguousarray(b2.reshape(2, 1)),
        "ones2": ones2, "bankT": bankT,
        "biasc": np.ascontiguousarray(bias.reshape(C, 1)), "ident": ident,
    }
    in_maps = [dict(common, x=x[i * BC:(i + 1) * BC]) for i in range(NCORES)]
    res = run_bass_kernel_spmd(nc, in_maps, core_ids=list(range(NCORES)))
    _cache["last_result"] = res
    out = np.concatenate([np.asarray(res.results[i]["out"]) for i in range(NCORES)],
                         axis=0)
    return out
